# revision 6
# baseline (speedup 1.0000x reference)
"""CFG dual self-attention kernel for 8 Trainium2 NeuronCores.

Strategy (tensor parallel on heads):
  - h = concat(hidden_cond, hidden_uncond) -> [4096 tokens, 5120]; host
    pre-transposes to hT [5120, 4096] so the QKV matmul contraction dim (5120)
    lands on SBUF partitions.
  - Each core owns 5 heads (640 of the 5120 q/k/v channels).  It computes
    qT/kT [640, 4096] (transposed layout: head-dim on partitions) and
    v [4096, 640] (natural layout) from hT with fp32r matmuls.
  - RMSNorm over the full 5120 dims needs a cross-core sum of squares:
    partial ssq per token is computed with ones-matmuls on the PE and
    allreduced across the 8 cores (32 KB collective, hidden under the V
    projection).
  - Attention per (batch, head) in scores-transposed layout
    scoresT[st, sq] = (rope(k) slice)^T @ rope(q): softmax denominators via
    ones-matmul column sums (interleaved PSUM accumulation groups), exp on
    the scalar engine, A@V accumulated with v-chunks stationary, and the
    1/colsum normalization folded into the PSUM->SBUF eviction of attn_outT.
  - Output projection: partial_out = attn_outT^T @ Wout[rows of this core]
    -> [4096, 5120] per-core partial; host sums the 8 partials (+ bout).
"""

import numpy as np

import concourse.bass as bass  # noqa: F401  (bass types via bacc)
import concourse.mybir as mybir
import concourse.tile as tile
from concourse import bacc
from concourse.bass_utils import run_bass_kernel_spmd

F32 = mybir.dt.float32
F32R = mybir.dt.float32r

NCORES = 8
EPS = 1e-6


def _vblocks(cw):
    # split the v output width into matmul N-blocks (>=256 keeps fp32r fast)
    if cw % 320 == 0:
        return [320] * (cw // 320)
    return [cw]


def build_program(S, DIM, H):
    """Emit the per-core bass program (identical on all cores; per-core data
    differences come entirely from the input tensors)."""
    HD = 128
    assert DIM == H * HD
    HPC = H // NCORES          # heads per core
    CW = HPC * HD              # per-core channel width for q/k/v
    CT = HPC                   # 128-col tiles per group
    NT = 2 * S                 # tokens across both batches
    DC = DIM // 128            # contraction chunks
    TBS = 256                  # token block in phase 1
    NTB = NT // TBS
    SQB = min(512, S)          # sq block in attention
    NSQ = S // SQB
    NST = S // 128             # st (key) chunks per batch
    VNB = _vblocks(CW)
    ONB = DIM // 512           # out-proj N blocks

    nc = bacc.Bacc("TRN2", target_bir_lowering=False, debug=False,
                   num_devices=NCORES)

    hT = nc.dram_tensor("hT", [DIM, NT], F32, kind="ExternalInput")
    wq = nc.dram_tensor("wq", [DIM, CW], F32, kind="ExternalInput")
    wk = nc.dram_tensor("wk", [DIM, CW], F32, kind="ExternalInput")
    wv = nc.dram_tensor("wv", [DIM, CW], F32, kind="ExternalInput")
    bq = nc.dram_tensor("bq", [128, CT], F32, kind="ExternalInput")
    bk = nc.dram_tensor("bk", [128, CT], F32, kind="ExternalInput")
    bv = nc.dram_tensor("bv", [1, CW], F32, kind="ExternalInput")
    wqn = nc.dram_tensor("wqn", [128, CT], F32, kind="ExternalInput")
    wkn = nc.dram_tensor("wkn", [128, CT], F32, kind="ExternalInput")
    cosT = nc.dram_tensor("cosT", [128, S], F32, kind="ExternalInput")
    sinrT = nc.dram_tensor("sinrT", [128, S], F32, kind="ExternalInput")
    wout = nc.dram_tensor("wout", [CW, DIM], F32, kind="ExternalInput")
    outp = nc.dram_tensor("outp", [NT, DIM], F32, kind="ExternalOutput")

    with tile.TileContext(nc) as tc:
        with (
            tc.tile_pool(name="dram", bufs=1, space="DRAM") as dram,
            tc.tile_pool(name="persist", bufs=1) as persist,
        ):
            qsc = dram.tile([CW, NT], F32, tag="qsc")
            ksc = dram.tile([CW, NT], F32, tag="ksc")
            vsc = dram.tile([NT, CW], F32, tag="vsc")
            aosc = dram.tile([CW, NT], F32, tag="aosc")
            cc_in = dram.tile([2, NT], F32, tag="cc_in")
            cc_out = dram.tile([2, NT], F32, tag="cc_out")

            # constants
            ones_f = persist.tile([128, 1], F32, tag="ones_f")
            nc.vector.memset(ones_f[:], 1.0)
            ones = persist.tile([128, 1], F32R, tag="ones")
            nc.vector.tensor_copy(ones[:], ones_f[:])

            bq_t = persist.tile([128, CT], F32, tag="bq")
            nc.sync.dma_start(bq_t[:], bq[:])
            bk_t = persist.tile([128, CT], F32, tag="bk")
            nc.sync.dma_start(bk_t[:], bk[:])
            bv_row = persist.tile([1, CW], F32, tag="bv_row")
            nc.sync.dma_start(bv_row[:], bv[:])
            bv_bc = persist.tile([128, CW], F32, tag="bv_bc")
            nc.gpsimd.partition_broadcast(bv_bc[:], bv_row[:])
            wqn_t = persist.tile([128, CT], F32, tag="wqn")
            nc.sync.dma_start(wqn_t[:], wqn[:])
            wkn_t = persist.tile([128, CT], F32, tag="wkn")
            nc.sync.dma_start(wkn_t[:], wkn[:])

            # ---------------- phase 1: qkv projections + ssq partials -------
            with (
                tc.tile_pool(name="wp", bufs=1) as wp,
                tc.tile_pool(name="hp", bufs=1) as hp,
                tc.tile_pool(name="ev", bufs=4) as evp,
                tc.tile_pool(name="st1", bufs=2) as st1,
            ):
                for gi, (wdram, bias_t, spill) in enumerate(
                    [(wq, bq_t, qsc), (wk, bk_t, ksc)]
                ):
                    with (
                        tc.tile_pool(name=f"ps{gi}", bufs=4, space="PSUM") as psp,
                        tc.tile_pool(name=f"sq{gi}", bufs=2, space="PSUM") as sqp,
                    ):
                        wall = wp.tile([128, DC, CW], F32R, tag="wall")
                        for ch in range(DC):
                            nc.sync.dma_start(
                                wall[:, ch, :],
                                wdram[ch * 128:(ch + 1) * 128, :].bitcast(F32R))
                        for tb in range(NTB):
                            hall = hp.tile([128, DC, TBS], F32R, tag="hall")
                            for ch in range(DC):
                                nc.sync.dma_start(
                                    hall[:, ch, :],
                                    hT[ch * 128:(ch + 1) * 128,
                                       tb * TBS:(tb + 1) * TBS].bitcast(F32R))
                            ssq_ps = sqp.tile([1, TBS], F32, tag="ssq")
                            for ct in range(CT):
                                pq = psp.tile([128, TBS], F32, tag="acc")
                                for ch in range(DC):
                                    nc.tensor.matmul(
                                        pq[:],
                                        wall[:, ch, ct * 128:(ct + 1) * 128],
                                        hall[:, ch, :],
                                        start=(ch == 0), stop=(ch == DC - 1))
                                evsq = evp.tile([128, 2, TBS], F32R, tag="evsq")
                                nc.vector.tensor_scalar_add(
                                    evsq[:, 0, :], pq[:], bias_t[:, ct:ct + 1])
                                nc.sync.dma_start(
                                    spill[ct * 128:(ct + 1) * 128,
                                          tb * TBS:(tb + 1) * TBS]
                                    .bitcast(F32R), evsq[:, 0, :])
                                nc.vector.tensor_mul(
                                    evsq[:, 1, :], evsq[:, 0, :].bitcast(F32),
                                    evsq[:, 0, :].bitcast(F32))
                                nc.tensor.matmul(
                                    ssq_ps[:], ones[:], evsq[:, 1, :],
                                    start=(ct == 0), stop=(ct == CT - 1))
                            stg = st1.tile([1, TBS], F32, tag="stg")
                            nc.vector.tensor_copy(stg[:], ssq_ps[:])
                            nc.sync.dma_start(
                                cc_in[gi:gi + 1, tb * TBS:(tb + 1) * TBS],
                                stg[:])

                # allreduce the ssq partials (overlaps with the v group below)
                nc.gpsimd.collective_compute(
                    "AllReduce", mybir.AluOpType.add,
                    replica_groups=[list(range(NCORES))],
                    ins=[cc_in[:].opt()], outs=[cc_out[:].opt()])

                # v projection (natural layout, hT token-tiles stationary)
                with tc.tile_pool(name="psv", bufs=2, space="PSUM") as psv:
                    wall = wp.tile([128, DC, CW], F32R, tag="wall")
                    for ch in range(DC):
                        nc.sync.dma_start(
                            wall[:, ch, :],
                            wv[ch * 128:(ch + 1) * 128, :].bitcast(F32R))
                    for tb in range(NTB):
                        hall = hp.tile([128, DC, TBS], F32R, tag="hall")
                        for ch in range(DC):
                            nc.sync.dma_start(
                                hall[:, ch, :],
                                hT[ch * 128:(ch + 1) * 128,
                                   tb * TBS:(tb + 1) * TBS].bitcast(F32R))
                        nsub = TBS // 128
                        pv = [[psv.tile([128, nb], F32, tag=f"pv{ts}_{i}",
                                        name=f"pv{ts}_{i}")
                               for i, nb in enumerate(VNB)]
                              for ts in range(nsub)]
                        for ch in range(DC):
                            for ts in range(nsub):
                                off = 0
                                for i, nb in enumerate(VNB):
                                    nc.tensor.matmul(
                                        pv[ts][i][:],
                                        hall[:, ch, ts * 128:(ts + 1) * 128],
                                        wall[:, ch, off:off + nb],
                                        start=(ch == 0), stop=(ch == DC - 1))
                                    off += nb
                        for ts in range(nsub):
                            off = 0
                            for i, nb in enumerate(VNB):
                                evv = evp.tile([128, max(VNB)], F32R, tag="evv")
                                nc.vector.tensor_add(
                                    evv[:, :nb], pv[ts][i][:],
                                    bv_bc[:, off:off + nb])
                                nc.sync.dma_start(
                                    vsc[tb * TBS + ts * 128:
                                        tb * TBS + (ts + 1) * 128,
                                        off:off + nb].bitcast(F32R),
                                    evv[:, :nb])
                                off += nb

            # ---------------- phase 2: rinv = 1/sqrt(mean ssq + eps) --------
            # transient [1, NT] row tiles; results live in 4 broadcast tiles
            with (
                tc.tile_pool(name="rrow", bufs=1) as rrow,
                tc.tile_pool(name="rbp", bufs=1) as rbp,
            ):
                rb = {}
                for gi in range(2):
                    rg = rrow.tile([1, NT], F32, tag="rg")
                    nc.sync.dma_start(rg[:], cc_out[gi:gi + 1, :])
                    nc.vector.tensor_scalar(rg[:], rg[:], 1.0 / DIM, EPS,
                                            mybir.AluOpType.mult,
                                            mybir.AluOpType.add)
                    nc.scalar.activation(rg[:], rg[:],
                                         mybir.ActivationFunctionType.Sqrt)
                    nc.vector.reciprocal(rg[:], rg[:])
                    if gi == 0:
                        # fold the attention scale into the q-side factor
                        nc.vector.tensor_scalar_mul(rg[:], rg[:],
                                                    float(HD) ** -0.5)
                    for b in range(2):
                        t = rbp.tile([128, S], F32, tag=f"rb{gi}{b}",
                                     name=f"rb{gi}{b}")
                        nc.gpsimd.partition_broadcast(
                            t[:], rg[0:1, b * S:(b + 1) * S])
                        rb[(gi, b)] = t

                # ------------- phase 3: attention per (batch, head) ---------
                with (
                    tc.tile_pool(name="p3", bufs=2) as p3,
                    tc.tile_pool(name="p3e", bufs=3) as p3e,
                    tc.tile_pool(name="cs3", bufs=1) as cs3,
                    tc.tile_pool(name="ps_sc", bufs=2, space="PSUM") as ps_sc,
                    tc.tile_pool(name="ps_cs", bufs=2, space="PSUM") as ps_cs,
                    tc.tile_pool(name="ps_av", bufs=2, space="PSUM") as ps_av,
                ):
                    cosT_t = cs3.tile([128, S], F32, tag="cosT")
                    nc.sync.dma_start(cosT_t[:], cosT[:])
                    sinrT_t = cs3.tile([128, S], F32, tag="sinrT")
                    nc.sync.dma_start(sinrT_t[:], sinrT[:])

                    for b in range(2):
                        for hh in range(HPC):
                            qkr = []
                            for gi, (spill, wn) in enumerate(
                                    [(qsc, wqn_t), (ksc, wkn_t)]):
                                xt = p3.tile([128, S], F32, tag="xt")
                                nc.sync.dma_start(
                                    xt[:], spill[hh * 128:(hh + 1) * 128,
                                                 b * S:(b + 1) * S])
                                nc.vector.tensor_mul(xt[:], xt[:],
                                                     rb[(gi, b)][:])
                                nc.vector.tensor_scalar_mul(
                                    xt[:], xt[:], wn[:, hh:hh + 1])
                                tmc = p3.tile([128, S], F32, tag="tmc")
                                nc.vector.tensor_mul(tmc[:], xt[:], cosT_t[:])
                                tms = p3.tile([128, S], F32, tag="tms")
                                nc.vector.tensor_mul(
                                    tms[0:64, :], xt[64:128, :],
                                    sinrT_t[64:128, :])
                                nc.vector.tensor_mul(
                                    tms[64:128, :], xt[0:64, :],
                                    sinrT_t[0:64, :])
                                xr = p3.tile([128, S], F32R, tag="xr")
                                nc.vector.tensor_add(xr[:], tmc[:], tms[:])
                                qkr.append(xr)
                            qr, kr = qkr
                            vt = p3.tile([128, NST, 128], F32R, tag="vt")
                            nc.sync.dma_start(
                                vt[:], vsc[b * S:(b + 1) * S,
                                           hh * 128:(hh + 1) * 128]
                                .bitcast(F32R)
                                .rearrange("(c p) d -> p c d", p=128))
                            ao = p3.tile([128, S], F32R, tag="ao")
                            for sqb in range(NSQ):
                                cs = ps_cs.tile([1, SQB], F32, tag="cs")
                                av = ps_av.tile([128, SQB], F32, tag="av")
                                for st in range(NST):
                                    sc = ps_sc.tile([128, SQB], F32, tag="sc")
                                    nc.tensor.matmul(
                                        sc[:], kr[:, st * 128:(st + 1) * 128],
                                        qr[:, sqb * SQB:(sqb + 1) * SQB],
                                        start=True, stop=True)
                                    et = p3e.tile([128, SQB], F32R, tag="et")
                                    nc.scalar.activation(
                                        et[:], sc[:],
                                        mybir.ActivationFunctionType.Exp)
                                    nc.tensor.matmul(
                                        cs[:], ones[:], et[:],
                                        start=(st == 0), stop=(st == NST - 1))
                                    nc.tensor.matmul(
                                        av[:], vt[:, st, :], et[:],
                                        start=(st == 0), stop=(st == NST - 1))
                                rc = p3.tile([1, SQB], F32, tag="rc")
                                nc.vector.reciprocal(rc[:], cs[:])
                                rb2 = p3.tile([128, SQB], F32, tag="rb2")
                                nc.gpsimd.partition_broadcast(rb2[:], rc[:])
                                nc.vector.tensor_mul(
                                    ao[:, sqb * SQB:(sqb + 1) * SQB],
                                    av[:], rb2[:])
                            nc.sync.dma_start(
                                aosc[hh * 128:(hh + 1) * 128,
                                     b * S:(b + 1) * S].bitcast(F32R), ao[:])

            # ---------------- phase 4: partial output projection ------------
            with (
                tc.tile_pool(name="wo", bufs=1) as wo,
                tc.tile_pool(name="p4", bufs=4) as p4,
                tc.tile_pool(name="oe", bufs=4) as oep,
                tc.tile_pool(name="ps4", bufs=4, space="PSUM") as ps4,
            ):
                wot = wo.tile([128, HPC, DIM], F32R, tag="wot")
                for ch in range(HPC):
                    nc.sync.dma_start(
                        wot[:, ch, :],
                        wout[ch * 128:(ch + 1) * 128, :].bitcast(F32R))
                for tt in range(NT // 128):
                    aot = p4.tile([128, HPC, 128], F32R, tag="aot")
                    nc.sync.dma_start(
                        aot[:], aosc[:, tt * 128:(tt + 1) * 128].bitcast(F32R)
                        .rearrange("(c p) n -> p c n", p=128))
                    for nb in range(ONB):
                        po = ps4.tile([128, 512], F32, tag="po")
                        for ch in range(HPC):
                            nc.tensor.matmul(
                                po[:], aot[:, ch, :],
                                wot[:, ch, nb * 512:(nb + 1) * 512],
                                start=(ch == 0), stop=(ch == HPC - 1))
                        oe = oep.tile([128, 512], F32, tag="oe")
                        nc.vector.tensor_copy(oe[:], po[:])
                        nc.sync.dma_start(
                            outp[tt * 128:(tt + 1) * 128,
                                 nb * 512:(nb + 1) * 512], oe[:])
    nc.finalize()
    return nc


_PROGRAM_CACHE = {}


def _get_program(S, DIM, H):
    key = (S, DIM, H)
    if key not in _PROGRAM_CACHE:
        _PROGRAM_CACHE[key] = build_program(S, DIM, H)
    return _PROGRAM_CACHE[key]


def make_in_maps(S, DIM, H, hidden_cond, hidden_uncond, cos_freqs, sin_freqs,
                 Wqkv, bqkv, wq_norm, wk_norm, Wout, bout):
    HD = 128
    HPC = H // NCORES
    CW = HPC * HD
    h = np.concatenate([np.asarray(hidden_cond), np.asarray(hidden_uncond)],
                       axis=0).reshape(2 * S, DIM)
    hT = np.ascontiguousarray(h.T)
    cosT = np.ascontiguousarray(np.asarray(cos_freqs).T)
    sinT = np.asarray(sin_freqs).T  # [128, S]
    HF = HD // 2
    sinrT = np.concatenate([sinT[HF:], -sinT[:HF]], axis=0)
    sinrT = np.ascontiguousarray(sinrT)
    Wqkv = np.asarray(Wqkv)
    bqkv = np.asarray(bqkv)
    wq_norm = np.asarray(wq_norm)
    wk_norm = np.asarray(wk_norm)
    Wout = np.asarray(Wout)

    in_maps = []
    for c in range(NCORES):
        sl = slice(c * CW, (c + 1) * CW)
        bq_c = bqkv[0 * DIM:1 * DIM][sl].reshape(HPC, HD).T
        bk_c = bqkv[1 * DIM:2 * DIM][sl].reshape(HPC, HD).T
        bv_c = bqkv[2 * DIM:3 * DIM][sl].reshape(1, CW)
        in_maps.append({
            "hT": hT,
            "wq": np.ascontiguousarray(Wqkv[:, 0 * DIM:1 * DIM][:, sl]),
            "wk": np.ascontiguousarray(Wqkv[:, 1 * DIM:2 * DIM][:, sl]),
            "wv": np.ascontiguousarray(Wqkv[:, 2 * DIM:3 * DIM][:, sl]),
            "bq": np.ascontiguousarray(bq_c),
            "bk": np.ascontiguousarray(bk_c),
            "bv": np.ascontiguousarray(bv_c),
            "wqn": np.ascontiguousarray(wq_norm[sl].reshape(HPC, HD).T),
            "wkn": np.ascontiguousarray(wk_norm[sl].reshape(HPC, HD).T),
            "cosT": cosT,
            "sinrT": sinrT,
            "wout": np.ascontiguousarray(Wout[sl, :]),
        })
    return in_maps


def run(S, DIM, H, inputs):
    nc = _get_program(S, DIM, H)
    in_maps = make_in_maps(S, DIM, H, **inputs)
    res = run_bass_kernel_spmd(nc, in_maps, list(range(NCORES)))
    partial = np.zeros((2 * S, DIM), np.float64)
    for r in res.results:
        partial += r["outp"].astype(np.float64)
    out = (partial + np.asarray(inputs["bout"])[None, :]).astype(np.float32)
    out = out.reshape(2, 1, S, DIM)
    return out[0], out[1]


def kernel(hidden_cond, hidden_uncond, cos_freqs, sin_freqs,
           Wqkv, bqkv, wq_norm, wk_norm, Wout, bout):
    B, S, DIM = np.asarray(hidden_cond).shape
    assert B == 1
    H = DIM // 128
    return run(S, DIM, H, dict(
        hidden_cond=hidden_cond, hidden_uncond=hidden_uncond,
        cos_freqs=cos_freqs, sin_freqs=sin_freqs, Wqkv=Wqkv, bqkv=bqkv,
        wq_norm=wq_norm, wk_norm=wk_norm, Wout=Wout, bout=bout))
